# revision 30
# baseline (speedup 1.0000x reference)
"""Trainium2 Bass kernel for nn_AC_Filter_PreNorm_Net (causal MHA, embed_dim=3,
L=2048, B=32) + post-attention integrator chain, data-parallel over 8 cores.

Key algebraic reduction (verified to ~2e-6 rel err vs the jax reference):
every op after the softmax attention (out-projection, the four
MaskedLinear+multiplicative-gate "velocity" layers, the three integrator
steps, and the final sigma rescale) is affine in the attention output.
The whole network therefore collapses to

    out^T[8, q] = (Mrow @ [N; D][:, q]) / D[q]

where N[3, q] / D[q] are the unnormalized softmax numerator/denominator.
Folding further, N and D come from one PSUM-accumulated matmul with
lhsT = [V @ M^T | 1] ("VM"), so the per-core device graph is just:

    scores^T = K Q^T   (TensorE, contraction K=3, bf16)
    E = exp(scores)    (ScalarE, fp32 PSUM -> bf16 SBUF)
                       [diagonal tiles masked on VectorE]
    acc = VM^T E       (TensorE bf16, fp32 PSUM accumulation, 9 live rows)

The device returns the 8 numerator rows + denominator row per position;
the final elementwise division (0.1% of the FLOPs) happens during the
host-side unshard, as does the [9, L] -> [L, 8] layout transpose.

Q^T, K^T (with feature-norm, 1/sqrt(3), and biases folded in) and VM are
tiny O(L*D^2) projections computed on the host; all O(L^2) work is on
device. B=32 is sharded 4 batches/core across 8 cores; no collectives.
bf16 end-to-end numerics measured at 1.8e-3 rel err (gate: 2e-2).
"""

import os
import sys
import math

import numpy as np
import ml_dtypes

BF16_NP = ml_dtypes.bfloat16

for _p in ("/opt/trn_rl_repo",):
    if os.path.isdir(_p) and _p not in sys.path:
        sys.path.append(_p)

import concourse.bacc as bacc
import concourse.tile as tile
from concourse import mybir
from concourse.bass_utils import run_bass_kernel_spmd

B, L, D = 32, 2048, 3
NCORES = 8
BPC = B // NCORES          # batches per core
QCH = 512                  # q-chunk width (one fp32 PSUM bank)
NQC = L // QCH
KTILE = 128                # keys per tile (partition dim)
NKT = L // KTILE
DT = 0.01
EPS = 1e-5
F32 = mybir.dt.float32
BF16 = mybir.dt.bfloat16

_built = None              # cached compiled Bass graph

# exec_time_ns of the last traced run (None unless BASS_KERNEL_TRACE=1)
LAST_EXEC_TIME_NS = None


def _build():
    from contextlib import ExitStack

    nc = bacc.Bacc("TRN2", target_bir_lowering=False, debug=False,
                   num_devices=NCORES)

    # VM has 65 columns (0-7 numerator rows, 32 the all-ones denominator
    # column, rest zero): both matmul shapes then round up to the full
    # 128x128 PE tile mode, which keeps the HAM activity monitor warm
    # (tiled modes run at the cold 1.2 GHz clock) and avoids mode switches
    qk_d = nc.dram_tensor("qk", [BPC, 65, 2, L], BF16,
                          kind="ExternalInput").ap()
    vm_d = nc.dram_tensor("vm", [BPC, 128, NKT, 65], BF16,
                          kind="ExternalInput").ap()
    mk_d = nc.dram_tensor("mask", [128, 128], BF16, kind="ExternalInput").ap()
    y_d = nc.dram_tensor("y", [BPC, 9, L], F32, kind="ExternalOutput").ap()

    with tile.TileContext(nc) as tc, ExitStack() as ctx:
        singles = ctx.enter_context(tc.tile_pool(name="singles", bufs=1))
        io_pool = ctx.enter_context(tc.tile_pool(name="io", bufs=2))
        e_pool = ctx.enter_context(tc.tile_pool(name="e", bufs=4))
        s_pool = ctx.enter_context(tc.tile_pool(name="s", bufs=3, space="PSUM"))
        acc_pool = ctx.enter_context(
            tc.tile_pool(name="acc", bufs=2, space="PSUM"))

        mask_sb = singles.tile([128, 128], BF16)
        nc.sync.dma_start(out=mask_sb[:], in_=mk_d[:])

        # dummy activation with no deps: pulls the ~2.7us exp-table load
        # to kernel start, overlapping the input DMAs
        warm = singles.tile([1, 8], F32)
        nc.vector.memset(warm[:], 0.0)
        nc.scalar.activation(warm[:], warm[:],
                             mybir.ActivationFunctionType.Exp)

        for b in range(BPC):
            qk_sb = io_pool.tile([65, 2, L], BF16, tag="qk")
            # split input DMAs so earlier q-chunks' data lands first
            for c0 in range(NQC):
                nc.sync.dma_start(out=qk_sb[:, :, c0 * QCH:(c0 + 1) * QCH],
                                  in_=qk_d[b][:, :, c0 * QCH:(c0 + 1) * QCH])
            vm_sb = io_pool.tile([128, NKT, 65], BF16, tag="vm")
            nc.sync.dma_start(out=vm_sb[:, 0:4, :], in_=vm_d[b][:, 0:4, :])
            nc.sync.dma_start(out=vm_sb[:, 4:8, :], in_=vm_d[b][:, 4:8, :])
            nc.sync.dma_start(out=vm_sb[:, 8:NKT, :], in_=vm_d[b][:, 8:NKT, :])
            out_sb = io_pool.tile([65, L], F32, tag="out")

            # last batch runs q-chunks big-to-small so the kernel tail is
            # the shortest dependency chain
            qc_order = range(NQC) if b < BPC - 1 else range(NQC - 1, -1, -1)
            for qc in qc_order:
                acc = acc_pool.tile([65, QCH], F32)
                n_kt = 4 * qc + 4      # causal: key tiles 0 .. 4*qc+3
                d0 = 4 * qc
                # groups of (kt, s-offset, width, acc-col-offset) sharing one
                # 2-bank PSUM tile and ONE gap-free exp each. Non-diagonal
                # tiles in pairs; the 4 diagonal staircase tiles pack
                # back-to-back as (j0,j1) -> 896 cols and (j2,j3) -> 384.
                groups = [[(2 * i, 0, QCH, 0), (2 * i + 1, QCH, QCH, 0)]
                          for i in range(2 * qc)]
                groups.append([(d0 + 0, 0, 512, 0),
                               (d0 + 1, 512, 384, 128)])
                groups.append([(d0 + 2, 0, 256, 256),
                               (d0 + 3, 256, 128, 384)])
                pv_idx = 0
                for group in groups:
                    s = s_pool.tile([128, 2 * QCH], F32)
                    for kt, soff, w, co in group:
                        nc.tensor.matmul(
                            s[:, soff:soff + w],
                            lhsT=qk_sb[:, 1, kt * KTILE:(kt + 1) * KTILE],
                            rhs=qk_sb[:, 0, qc * QCH + co:(qc + 1) * QCH],
                            start=True, stop=True)
                    e = e_pool.tile([128, 2 * QCH], BF16)
                    hi = group[-1][1] + group[-1][2]
                    nc.scalar.activation(
                        e[:, 0:hi], s[:, 0:hi],
                        mybir.ActivationFunctionType.Exp)
                    for kt, soff, w, co in group:
                        if kt >= d0:          # diagonal tile: triangular mask
                            nc.vector.tensor_mul(
                                e[:, soff:soff + 128], e[:, soff:soff + 128],
                                mask_sb[:])
                    for kt, soff, w, co in group:
                        nc.tensor.matmul(
                            acc[:, co:QCH],
                            lhsT=vm_sb[:, kt, :],
                            rhs=e[:, soff:soff + w],
                            start=(pv_idx == 0), stop=(pv_idx == n_kt - 1))
                        pv_idx += 1

                nc.vector.tensor_copy(
                    out_sb[:, qc * QCH:(qc + 1) * QCH], acc[:])
                # per-chunk output DMAs on the (idle) gpsimd queue so they
                # never block the next batch's input DMAs on the sync queue
                nc.gpsimd.dma_start(
                    out=y_d[b, 0:8, qc * QCH:(qc + 1) * QCH],
                    in_=out_sb[0:8, qc * QCH:(qc + 1) * QCH])
                nc.gpsimd.dma_start(
                    out=y_d[b, 8:9, qc * QCH:(qc + 1) * QCH],
                    in_=out_sb[32:33, qc * QCH:(qc + 1) * QCH])

    nc.compile()
    return nc


def _host_prep(inputs):
    """Fold the network's parameters into q/k projections and the VM matrix,
    and build per-core device inputs."""
    x = np.asarray(inputs["inputs"], dtype=np.float32)          # [B, L, 3]
    Wi = np.asarray(inputs["in_proj_w"], dtype=np.float64)      # [9, 3]
    bi = np.asarray(inputs["in_proj_b"], dtype=np.float64)      # [9]
    Wo = np.asarray(inputs["out_proj_w"], dtype=np.float64)     # [3, 3]
    bo = np.asarray(inputs["out_proj_b"], dtype=np.float64)     # [3]
    sigma = np.asarray(inputs["sigma"], dtype=np.float64)       # [2]
    f1_w = np.asarray(inputs["f1_w"], dtype=np.float64)
    f1_b = np.asarray(inputs["f1_b"], dtype=np.float64)
    f2_w = np.asarray(inputs["f2_w"], dtype=np.float64)
    f2_b = np.asarray(inputs["f2_b"], dtype=np.float64)
    g1_w = np.asarray(inputs["g1_w"], dtype=np.float64)
    g1_b = np.asarray(inputs["g1_b"], dtype=np.float64)
    g2_w = np.asarray(inputs["g2_w"], dtype=np.float64)
    g2_b = np.asarray(inputs["g2_b"], dtype=np.float64)
    m1 = float(np.asarray(inputs["m1_s"]))
    m2 = float(np.asarray(inputs["m2_s"]))

    scale = sigma + EPS
    dvec = np.array([1.0, 1.0 / scale[0], 1.0 / scale[1]])
    s3 = math.sqrt(3.0)

    Wq, Wk, Wv = Wi[0:3], Wi[3:6], Wi[6:9]
    bq, bk, bv = bi[0:3], bi[3:6], bi[6:9]
    Wq_eff = (Wq * dvec[None, :]) / s3
    bq_eff = bq / s3
    Wk_eff = Wk * dvec[None, :]
    bk_eff = bk
    Wv_eff = Wv * dvec[None, :]
    bv_eff = bv

    # affine collapse of the post-attention network: states are affine in
    # u = [1, a1, a2] (a = attention output channels 1, 2)
    e1 = np.array([1.0, 0.0, 0.0])

    def G(P):
        r1 = m1 * (g1_w @ P + g1_b[:, None] * e1[None, :])
        r2 = m2 * (g2_w @ P + g2_b[:, None] * e1[None, :])
        return np.vstack([np.zeros((1, 3)), r1, r2])

    P1 = np.eye(3)
    P2 = P1 + DT * G(P1)
    P3 = P2 + DT * G(P2)
    P4 = P3 + DT * G(P3)
    r7 = P4[1, :] + DT * m1 * (f1_w @ P4 + f1_b[:, None] * e1[None, :])[0]
    r8 = P4[2, :] + DT * m2 * (f2_w @ P4 + f2_b[:, None] * e1[None, :])[0]
    A = np.vstack([
        scale[0] * P2[1, :], scale[1] * P2[2, :],
        scale[0] * P3[1, :], scale[1] * P3[2, :],
        scale[0] * P4[1, :], scale[1] * P4[2, :],
        scale[0] * r7, scale[1] * r8,
    ])                                                  # [8, 3] in u-space
    U = np.zeros((3, 4))                                # u = U @ [ctx; 1]
    U[0, 3] = 1.0
    U[1, 0:3] = Wo[1, :]
    U[1, 3] = bo[1]
    U[2, 0:3] = Wo[2, :]
    U[2, 3] = bo[2]
    M = A @ U                                           # [8, 4]

    # VM: per-key row [ (V_ext @ M^T)[k], 1 ]  with V_ext = [V | 1]
    WvT_ext = np.zeros((4, 4))
    WvT_ext[0:3, 0:3] = Wv_eff.T
    WvT_ext[3, 0:3] = bv_eff
    WvT_ext[3, 3] = 1.0
    WVM = WvT_ext @ M.T                                 # [4, 8]
    WVM_ext = np.zeros((4, 65))
    WVM_ext[:, 0:8] = WVM
    WVM_ext[3, 32] = 1.0            # denominator column (partition 32)

    x_aug = np.concatenate([x, np.ones((B, L, 1), np.float32)], axis=-1)
    Wq_augT = np.concatenate([Wq_eff.T, bq_eff[None, :]],
                             axis=0).astype(np.float32)          # [4, 3]
    Wk_augT = np.concatenate([Wk_eff.T, bk_eff[None, :]],
                             axis=0).astype(np.float32)
    q_t = np.einsum("bld,dc->bcl", x_aug, Wq_augT)               # [B, 3, L]
    k_t = np.einsum("bld,dc->bcl", x_aug, Wk_augT)
    vm = x_aug @ WVM_ext.astype(np.float32)                      # [B, L, 65]

    qk_dev = np.zeros((B, 65, 2, L), dtype=BF16_NP)          # K padded to 65
    qk_dev[:, 0:3, 0, :] = q_t.astype(BF16_NP)
    qk_dev[:, 0:3, 1, :] = k_t.astype(BF16_NP)
    vm_dev = np.ascontiguousarray(
        vm.reshape(B, NKT, 128, 65).transpose(0, 2, 1, 3).astype(BF16_NP))
    mask = (np.arange(128)[None, :] >=
            np.arange(128)[:, None]).astype(BF16_NP)
    in_maps = []
    for c in range(NCORES):
        sl = slice(c * BPC, (c + 1) * BPC)
        in_maps.append({
            "qk": np.ascontiguousarray(qk_dev[sl]),
            "vm": np.ascontiguousarray(vm_dev[sl]),
            "mask": mask,
        })
    return in_maps


def kernel(**inputs) -> np.ndarray:
    global _built, LAST_EXEC_TIME_NS
    if _built is None:
        _built = _build()
    nc = _built

    in_maps = _host_prep(inputs)

    trace = os.environ.get("BASS_KERNEL_TRACE", "") == "1"
    res = run_bass_kernel_spmd(nc, in_maps, list(range(NCORES)), trace=trace)
    if trace:
        LAST_EXEC_TIME_NS = res.exec_time_ns

    y = np.concatenate([res.results[c]["y"] for c in range(NCORES)],
                       axis=0)                                   # [B, 9, L]
    num = y[:, 0:8, :]
    den = y[:, 8:9, :]
    out = (num / den).transpose(0, 2, 1)                         # [B, L, 8]
    return np.ascontiguousarray(out.astype(np.float32))


# revision 31
# speedup vs baseline: 1.1875x; 1.1875x over previous
"""Trainium2 Bass kernel for nn_AC_Filter_PreNorm_Net (causal MHA, embed_dim=3,
L=2048, B=32) + post-attention integrator chain, data-parallel over 8 cores.

Key algebraic reduction (verified to ~2e-6 rel err vs the jax reference):
every op after the softmax attention (out-projection, the four
MaskedLinear+multiplicative-gate "velocity" layers, the three integrator
steps, and the final sigma rescale) is affine in the attention output.
The whole network therefore collapses to

    out^T[8, q] = (Mrow @ [N; D][:, q]) / D[q]

where N[3, q] / D[q] are the unnormalized softmax numerator/denominator.
Folding further, N and D come from one PSUM-accumulated matmul with
lhsT = [V @ M^T | 1] ("VM"), so the per-core device graph is just:

    scores^T = K Q^T   (TensorE, contraction K=3, bf16)
    E = exp(scores)    (ScalarE, fp32 PSUM -> bf16 SBUF)
                       [diagonal tiles masked on VectorE]
    acc = VM^T E       (TensorE bf16, fp32 PSUM accumulation, 9 live rows)

The device returns the 8 numerator rows + denominator row per position;
the final elementwise division (0.1% of the FLOPs) happens during the
host-side unshard, as does the [9, L] -> [L, 8] layout transpose.

Q^T, K^T (with feature-norm, 1/sqrt(3), and biases folded in) and VM are
tiny O(L*D^2) projections computed on the host; all O(L^2) work is on
device. B=32 is sharded 4 batches/core across 8 cores; no collectives.
bf16 end-to-end numerics measured at 1.8e-3 rel err (gate: 2e-2).
"""

import os
import sys
import math

import numpy as np
import ml_dtypes

BF16_NP = ml_dtypes.bfloat16

for _p in ("/opt/trn_rl_repo",):
    if os.path.isdir(_p) and _p not in sys.path:
        sys.path.append(_p)

import concourse.bacc as bacc
import concourse.tile as tile
from concourse import mybir
from concourse.bass_utils import run_bass_kernel_spmd

B, L, D = 32, 2048, 3
NCORES = 8
BPC = B // NCORES          # batches per core
QCH = 512                  # q-chunk width (one fp32 PSUM bank)
NQC = L // QCH
KTILE = 128                # keys per tile (partition dim)
NKT = L // KTILE
DT = 0.01
EPS = 1e-5
F32 = mybir.dt.float32
BF16 = mybir.dt.bfloat16

_built = None              # cached compiled Bass graph

# exec_time_ns of the last traced run (None unless BASS_KERNEL_TRACE=1)
LAST_EXEC_TIME_NS = None


def _build():
    from contextlib import ExitStack

    nc = bacc.Bacc("TRN2", target_bir_lowering=False, debug=False,
                   num_devices=NCORES)

    # VM has 65 columns (0-7 numerator rows, 32 the all-ones denominator
    # column, rest zero): both matmul shapes then round up to the full
    # 128x128 PE tile mode, which keeps the HAM activity monitor warm
    # (tiled modes run at the cold 1.2 GHz clock) and avoids mode switches
    qk_d = nc.dram_tensor("qk", [BPC, 65, 2, L], BF16,
                          kind="ExternalInput").ap()
    vm_d = nc.dram_tensor("vm", [BPC, 128, NKT, 65], BF16,
                          kind="ExternalInput").ap()
    mk_d = nc.dram_tensor("mask", [128, 128], BF16, kind="ExternalInput").ap()
    y_d = nc.dram_tensor("y", [BPC, 9, L], F32, kind="ExternalOutput").ap()

    with tile.TileContext(nc) as tc, ExitStack() as ctx:
        singles = ctx.enter_context(tc.tile_pool(name="singles", bufs=1))
        io_pool = ctx.enter_context(tc.tile_pool(name="io", bufs=2))
        e_pool = ctx.enter_context(tc.tile_pool(name="e", bufs=4))
        s_pool = ctx.enter_context(tc.tile_pool(name="s", bufs=3, space="PSUM"))
        acc_pool = ctx.enter_context(
            tc.tile_pool(name="acc", bufs=2, space="PSUM"))

        mask_sb = singles.tile([128, 128], BF16)
        nc.sync.dma_start(out=mask_sb[:], in_=mk_d[:])

        # dummy activation with no deps: pulls the ~2.7us exp-table load
        # to kernel start, overlapping the input DMAs
        warm = singles.tile([1, 8], F32)
        nc.vector.memset(warm[:], 0.0)
        nc.scalar.activation(warm[:], warm[:],
                             mybir.ActivationFunctionType.Exp)

        for b in range(BPC):
            qk_sb = io_pool.tile([65, 2, L], BF16, tag="qk")
            # split input DMAs so the first q-chunk's data lands first
            nc.sync.dma_start(out=qk_sb[:, :, 0:QCH], in_=qk_d[b][:, :, 0:QCH])
            nc.sync.dma_start(out=qk_sb[:, :, QCH:L], in_=qk_d[b][:, :, QCH:L])
            vm_sb = io_pool.tile([128, NKT, 65], BF16, tag="vm")
            nc.sync.dma_start(out=vm_sb[:, 0:4, :], in_=vm_d[b][:, 0:4, :])
            nc.sync.dma_start(out=vm_sb[:, 4:NKT, :], in_=vm_d[b][:, 4:NKT, :])
            out_sb = io_pool.tile([65, L], F32, tag="out")

            # last batch runs q-chunks big-to-small so the kernel tail is
            # the shortest dependency chain
            qc_order = range(NQC) if b < BPC - 1 else range(NQC - 1, -1, -1)
            for qc in qc_order:
                acc = acc_pool.tile([65, QCH], F32)
                n_kt = 4 * qc + 4      # causal: key tiles 0 .. 4*qc+3
                d0 = 4 * qc
                # groups of (kt, s-offset, width, acc-col-offset) sharing one
                # 2-bank PSUM tile and ONE gap-free exp each. Non-diagonal
                # tiles in pairs; the 4 diagonal staircase tiles pack
                # back-to-back as (j0,j1) -> 896 cols and (j2,j3) -> 384.
                groups = [[(2 * i, 0, QCH, 0), (2 * i + 1, QCH, QCH, 0)]
                          for i in range(2 * qc)]
                groups.append([(d0 + 0, 0, 512, 0),
                               (d0 + 1, 512, 384, 128)])
                groups.append([(d0 + 2, 0, 256, 256),
                               (d0 + 3, 256, 128, 384)])
                pv_idx = 0
                for group in groups:
                    s = s_pool.tile([128, 2 * QCH], F32)
                    for kt, soff, w, co in group:
                        nc.tensor.matmul(
                            s[:, soff:soff + w],
                            lhsT=qk_sb[:, 1, kt * KTILE:(kt + 1) * KTILE],
                            rhs=qk_sb[:, 0, qc * QCH + co:(qc + 1) * QCH],
                            start=True, stop=True)
                    e = e_pool.tile([128, 2 * QCH], BF16)
                    hi = group[-1][1] + group[-1][2]
                    nc.scalar.activation(
                        e[:, 0:hi], s[:, 0:hi],
                        mybir.ActivationFunctionType.Exp)
                    for kt, soff, w, co in group:
                        if kt >= d0:          # diagonal tile: triangular mask
                            nc.vector.tensor_mul(
                                e[:, soff:soff + 128], e[:, soff:soff + 128],
                                mask_sb[:])
                    for kt, soff, w, co in group:
                        nc.tensor.matmul(
                            acc[:, co:QCH],
                            lhsT=vm_sb[:, kt, :],
                            rhs=e[:, soff:soff + w],
                            start=(pv_idx == 0), stop=(pv_idx == n_kt - 1))
                        pv_idx += 1

                nc.vector.tensor_copy(
                    out_sb[:, qc * QCH:(qc + 1) * QCH], acc[:])
                # per-chunk output DMAs on the (idle) gpsimd queue so they
                # never block the next batch's input DMAs on the sync queue
                nc.gpsimd.dma_start(
                    out=y_d[b, 0:8, qc * QCH:(qc + 1) * QCH],
                    in_=out_sb[0:8, qc * QCH:(qc + 1) * QCH])
                nc.gpsimd.dma_start(
                    out=y_d[b, 8:9, qc * QCH:(qc + 1) * QCH],
                    in_=out_sb[32:33, qc * QCH:(qc + 1) * QCH])

    nc.compile()
    return nc


def _host_prep(inputs):
    """Fold the network's parameters into q/k projections and the VM matrix,
    and build per-core device inputs."""
    x = np.asarray(inputs["inputs"], dtype=np.float32)          # [B, L, 3]
    Wi = np.asarray(inputs["in_proj_w"], dtype=np.float64)      # [9, 3]
    bi = np.asarray(inputs["in_proj_b"], dtype=np.float64)      # [9]
    Wo = np.asarray(inputs["out_proj_w"], dtype=np.float64)     # [3, 3]
    bo = np.asarray(inputs["out_proj_b"], dtype=np.float64)     # [3]
    sigma = np.asarray(inputs["sigma"], dtype=np.float64)       # [2]
    f1_w = np.asarray(inputs["f1_w"], dtype=np.float64)
    f1_b = np.asarray(inputs["f1_b"], dtype=np.float64)
    f2_w = np.asarray(inputs["f2_w"], dtype=np.float64)
    f2_b = np.asarray(inputs["f2_b"], dtype=np.float64)
    g1_w = np.asarray(inputs["g1_w"], dtype=np.float64)
    g1_b = np.asarray(inputs["g1_b"], dtype=np.float64)
    g2_w = np.asarray(inputs["g2_w"], dtype=np.float64)
    g2_b = np.asarray(inputs["g2_b"], dtype=np.float64)
    m1 = float(np.asarray(inputs["m1_s"]))
    m2 = float(np.asarray(inputs["m2_s"]))

    scale = sigma + EPS
    dvec = np.array([1.0, 1.0 / scale[0], 1.0 / scale[1]])
    s3 = math.sqrt(3.0)

    Wq, Wk, Wv = Wi[0:3], Wi[3:6], Wi[6:9]
    bq, bk, bv = bi[0:3], bi[3:6], bi[6:9]
    Wq_eff = (Wq * dvec[None, :]) / s3
    bq_eff = bq / s3
    Wk_eff = Wk * dvec[None, :]
    bk_eff = bk
    Wv_eff = Wv * dvec[None, :]
    bv_eff = bv

    # affine collapse of the post-attention network: states are affine in
    # u = [1, a1, a2] (a = attention output channels 1, 2)
    e1 = np.array([1.0, 0.0, 0.0])

    def G(P):
        r1 = m1 * (g1_w @ P + g1_b[:, None] * e1[None, :])
        r2 = m2 * (g2_w @ P + g2_b[:, None] * e1[None, :])
        return np.vstack([np.zeros((1, 3)), r1, r2])

    P1 = np.eye(3)
    P2 = P1 + DT * G(P1)
    P3 = P2 + DT * G(P2)
    P4 = P3 + DT * G(P3)
    r7 = P4[1, :] + DT * m1 * (f1_w @ P4 + f1_b[:, None] * e1[None, :])[0]
    r8 = P4[2, :] + DT * m2 * (f2_w @ P4 + f2_b[:, None] * e1[None, :])[0]
    A = np.vstack([
        scale[0] * P2[1, :], scale[1] * P2[2, :],
        scale[0] * P3[1, :], scale[1] * P3[2, :],
        scale[0] * P4[1, :], scale[1] * P4[2, :],
        scale[0] * r7, scale[1] * r8,
    ])                                                  # [8, 3] in u-space
    U = np.zeros((3, 4))                                # u = U @ [ctx; 1]
    U[0, 3] = 1.0
    U[1, 0:3] = Wo[1, :]
    U[1, 3] = bo[1]
    U[2, 0:3] = Wo[2, :]
    U[2, 3] = bo[2]
    M = A @ U                                           # [8, 4]

    # VM: per-key row [ (V_ext @ M^T)[k], 1 ]  with V_ext = [V | 1]
    WvT_ext = np.zeros((4, 4))
    WvT_ext[0:3, 0:3] = Wv_eff.T
    WvT_ext[3, 0:3] = bv_eff
    WvT_ext[3, 3] = 1.0
    WVM = WvT_ext @ M.T                                 # [4, 8]
    WVM_ext = np.zeros((4, 65))
    WVM_ext[:, 0:8] = WVM
    WVM_ext[3, 32] = 1.0            # denominator column (partition 32)

    x_aug = np.concatenate([x, np.ones((B, L, 1), np.float32)], axis=-1)
    Wq_augT = np.concatenate([Wq_eff.T, bq_eff[None, :]],
                             axis=0).astype(np.float32)          # [4, 3]
    Wk_augT = np.concatenate([Wk_eff.T, bk_eff[None, :]],
                             axis=0).astype(np.float32)
    q_t = np.einsum("bld,dc->bcl", x_aug, Wq_augT)               # [B, 3, L]
    k_t = np.einsum("bld,dc->bcl", x_aug, Wk_augT)
    vm = x_aug @ WVM_ext.astype(np.float32)                      # [B, L, 65]

    qk_dev = np.zeros((B, 65, 2, L), dtype=BF16_NP)          # K padded to 65
    qk_dev[:, 0:3, 0, :] = q_t.astype(BF16_NP)
    qk_dev[:, 0:3, 1, :] = k_t.astype(BF16_NP)
    vm_dev = np.ascontiguousarray(
        vm.reshape(B, NKT, 128, 65).transpose(0, 2, 1, 3).astype(BF16_NP))
    mask = (np.arange(128)[None, :] >=
            np.arange(128)[:, None]).astype(BF16_NP)
    in_maps = []
    for c in range(NCORES):
        sl = slice(c * BPC, (c + 1) * BPC)
        in_maps.append({
            "qk": np.ascontiguousarray(qk_dev[sl]),
            "vm": np.ascontiguousarray(vm_dev[sl]),
            "mask": mask,
        })
    return in_maps


def kernel(**inputs) -> np.ndarray:
    global _built, LAST_EXEC_TIME_NS
    if _built is None:
        _built = _build()
    nc = _built

    in_maps = _host_prep(inputs)

    trace = os.environ.get("BASS_KERNEL_TRACE", "") == "1"
    res = run_bass_kernel_spmd(nc, in_maps, list(range(NCORES)), trace=trace)
    if trace:
        LAST_EXEC_TIME_NS = res.exec_time_ns

    y = np.concatenate([res.results[c]["y"] for c in range(NCORES)],
                       axis=0)                                   # [B, 9, L]
    num = y[:, 0:8, :]
    den = y[:, 8:9, :]
    out = (num / den).transpose(0, 2, 1)                         # [B, L, 8]
    return np.ascontiguousarray(out.astype(np.float32))
